# revision 15
# baseline (speedup 1.0000x reference)
"""MoE-routed conv kernel (Channel_Embedding ablation) for 8 trn2 NeuronCores.

Math (see reference):
  gates  = top2-renormalized softmax( x[:, :, -6:-1].reshape(B, D*5) @ w_gate )
  h      = tanh(conv1d(x, conv1_w, VALID) + conv1_b)            # [B, OC, L-2]
  out    = conv1d(h, conv2_w, 1x1) + conv2_b                    # [B, OC*E, L-2]
  y[b,oc,t] = sum_e gates[b,e] * out[b, oc*E+e, t]

Key algebraic fold: the expert combine commutes with the 1x1 conv, so per
batch element y[b] = W_eff[b] @ h[b] + b_eff[b] with
  W_eff[b] = sum_e gates[b,e] * conv2_w[:,:,0][oc*E+e, :],
  b_eff[b] = sum_e gates[b,e] * conv2_b[oc*E+e].

Sharding: data-parallel over batch B=32 across 8 cores (4 each, b=2q+p).

v3 design (trace-driven; see v1/v2 history in git-less comments):
  * Host pre-packs x into a bf16 [128, 8*1028] image whose column blocks
    interleave the two batch pairs per 512-position tile, so each tile's
    conv runs p0 into PSUM rows 0:64 and p1 into rows 64:128 of ONE bank
    => ONE [128, n] tanh per tile (ACT cost is per-column, so pairing
    halves activation work vs per-pair tanh).
  * The gating window is duplicated into the f32 const image: gating
    runs in exact f32 and never touches the bf16 x image.
  * NO W_eff DRAM bounce (v2's combine stalled ~12us on that DMA chain).
    Instead: gates are block-broadcast to a [128, 8] column via two tiny
    indicator matmuls, and the combine lhsT+bias image WB[128, 33] is
    built on the DVE as sum_e gcol[:, e] * cwb[:, 33e:33e+33] from a
    host-packed per-expert image. WB rows (p, q, *) hold W_eff[2q+p].T
    diag-blocks (cols 0:32) and b_eff (col 32).
  * Combine per tile = 4 CONCURRENT 32x32 sub-matmuls via tile_position
    (32i, 32i) -- span of one matmul for all four batches.
  * y accumulates in a [128=(p,q,oc2), LP] f32 image; stores are 4 plain
    DMAs with a 4D DRAM-side AP, overlapped with the loop.
  * DMA rings: x chunks on sync, consts + stores on scalar; only 11
    dma_starts total (v1 had 40, all on sync, which serialized issue).
"""

from contextlib import ExitStack

import numpy as np

import concourse.bacc as bacc
import concourse.mybir as mybir
import concourse.tile as tile
from concourse import bass_utils

B, D, L = 32, 64, 4096
E, TOPK, OC = 8, 2, 32
LP = L - 2  # 4094 valid conv outputs
NCORES = 8
NB = B // NCORES  # batch elements per core
TS = 512  # position tile (one PSUM bank of fp32)
NT = (LP + TS - 1) // TS
BW = 2 + TS  # x image block half-width (conv needs +2 cols)
XW = NT * 2 * BW  # x image width: per tile, p0 block then p1 block

BF16 = mybir.dt.bfloat16
FAST_DT = mybir.dt.float32r

# cf1 [128, NCF1] f32 column map
C_GW = 0  # [*, 10] gate window, col = 2t+p, row = 64q+d
C_WG = 10  # [*, 40] w_gate, col = 8t+e (dup in both q halves)
C_B1 = 50  # [*, 1] conv1 bias tiled 4x, row = 32j+oc
C_IND = 51  # [0:2, 256] block-broadcast indicators, 128 per q
C_CWB = C_IND + 256  # [*, 264] cwb[64p+32q+r, 33e+j]: conv2 expert image
NCF1 = C_CWB + 33 * E
NW1 = 3 * 2 * OC  # cw1 [128, 192] bf16 block-diag conv1 lhsT

_CACHE: dict = {}


def _softmax_top2(nc, sm, lg, f32, AX, OP, AF, q):
    """Per-half gating: lg [2, E] logits (PSUM) -> gates [2, E] in SBUF.

    gates = (e >= m2) * e / (m1 + m2 + 1e-6 * sum(e)), e = exp(logits) --
    identical to softmax -> top2 -> vk/(sum vk + 1e-6) in exact arithmetic.
    Returns gpad [32, 32] with gates for batches {2q, 2q+1} at [0:2, 0:E].
    """
    e_sb = sm.tile([2, E], f32, name=f"e_sb{q}")
    nc.scalar.activation(e_sb[:], lg[:], AF.Exp)
    m1 = sm.tile([2, 1], f32, name=f"m1_{q}")
    nc.vector.reduce_max(m1[:], e_sb[:], axis=AX.X)
    lt = sm.tile([2, E], f32, name=f"lt{q}")
    nc.vector.tensor_scalar(lt[:], e_sb[:], m1[:], None, op0=OP.is_lt)
    emsk = sm.tile([2, E], f32, name=f"emsk{q}")
    nc.vector.tensor_mul(emsk[:], lt[:], e_sb[:])  # e with the max zeroed
    m2 = sm.tile([2, 1], f32, name=f"m2_{q}")
    nc.vector.reduce_max(m2[:], emsk[:], axis=AX.X)
    den3 = sm.tile([2, 1], f32, name=f"den3{q}")
    nc.vector.tensor_add(den3[:], m1[:], m2[:])
    rcp = sm.tile([2, 1], f32, name=f"rcp{q}")
    nc.vector.reciprocal(rcp[:], den3[:])
    ge = sm.tile([2, E], f32, name=f"ge{q}")
    nc.vector.tensor_scalar(ge[:], e_sb[:], m2[:], None, op0=OP.is_ge)
    gnum = sm.tile([2, E], f32, name=f"gnum{q}")
    nc.vector.tensor_mul(gnum[:], ge[:], e_sb[:])
    gpad = sm.tile([32, 32], f32, name=f"gpad{q}")
    nc.vector.memset(gpad[:], 0.0)
    nc.vector.tensor_scalar(gpad[0:2, 0:E], gnum[:], rcp[:], None, op0=OP.mult)
    return gpad  # gpad[p, e] = gates[2q+p, e]


def _emit(ctx, tc, nc, xb_d, cf1_d, cw1_d, y_d):
    f32 = mybir.dt.float32
    AF = mybir.ActivationFunctionType
    AX = mybir.AxisListType
    OP = mybir.AluOpType

    const = ctx.enter_context(tc.tile_pool(name="const", bufs=1))
    sm = ctx.enter_context(tc.tile_pool(name="sm", bufs=1))
    psum_h = ctx.enter_context(tc.tile_pool(name="ph", bufs=3, space="PSUM"))
    psum_o = ctx.enter_context(tc.tile_pool(name="po", bufs=2, space="PSUM"))
    psum_s = ctx.enter_context(tc.tile_pool(name="ps", bufs=2, space="PSUM"))

    # ---- persistent images
    xb = const.tile([128, XW], BF16)
    cf1 = const.tile([128, NCF1], f32)
    cw1 = const.tile([128, NW1], BF16)
    WB = const.tile([128, 33], FAST_DT)  # W_eff.T diag blocks + b_eff col
    weTd = const.tile([128, 128], BF16)  # block-diag combine lhsT
    gcol = const.tile([128, E], f32)
    himg = const.tile([128, LP], BF16)  # h, row = 64p+32q+oc1
    yimg = const.tile([128, LP], f32)  # y, row = 64p+32q+oc2

    # ---- DMA issue: consts on the scalar ring, x chunks on the sync ring
    nc.scalar.dma_start(cf1[:], cf1_d.ap())
    nc.scalar.dma_start(cw1[:], cw1_d.ap())
    XCH = [0, BW, 2 * BW, 4 * BW, 6 * BW, 8 * BW, 10 * BW, 12 * BW, 16 * BW]
    for a0, a1 in zip(XCH[:-1], XCH[1:]):
        nc.sync.dma_start(xb[:, a0:a1], xb_d.ap()[:, a0:a1])

    # ---- ACT table warmup (exp/tanh share one table set; load it early)
    warm = sm.tile([1, 8], f32)
    nc.vector.memset(warm[:], 0.0)
    warm2 = sm.tile([1, 8], f32)
    nc.scalar.activation(warm2[:], warm[:], AF.Exp)

    # ---- PE warmup: bf16 dummy matmuls bridge PE activity from t=0 into
    # the first real matmuls so the power state ramps during the load.
    wsrc = sm.tile([128, 256], f32)
    nc.vector.memset(wsrc[:], 0.0)
    wsb = wsrc[:].bitcast(BF16)  # [128, 512] of zeros
    for _ in range(4):
        wup = psum_h.tile([128, TS], f32, tag="hp")
        nc.tensor.matmul(wup[:], wsb[:, 0:128], wsb[:], start=True, stop=True)

    # ---- gating from the f32 const image (exact f32, batches b=2q+p)
    gpads = []
    for q in range(2):
        lg = psum_s.tile([2, E], f32, tag="s", name=f"lg{q}")
        for t in range(5):
            nc.tensor.matmul(
                lg[:],
                cf1[D * q : D * q + D, C_GW + 2 * t : C_GW + 2 * t + 2],
                cf1[D * q : D * q + D, C_WG + E * t : C_WG + E * t + E],
                start=(t == 0),
                stop=(t == 4),
            )
        gpads.append(_softmax_top2(nc, sm, lg, f32, AX, OP, AF, q))

    # ---- block-broadcast gates: gcol[64p+32q+r, e] = gates[2q+p, e]
    gcp = psum_s.tile([128, E], f32, tag="s")
    for q in range(2):
        nc.tensor.matmul(
            gcp[:],
            cf1[0:2, C_IND + 128 * q : C_IND + 128 * q + 128],
            gpads[q][0:2, 0:E],
            start=(q == 0),
            stop=(q == 1),
        )
    nc.vector.tensor_copy(gcol[:], gcp[:])

    # ---- WB = sum_e gcol[:, e] * cwb_e  (DVE build; no DRAM bounce)
    acc = sm.tile([128, 33], f32)
    nc.vector.tensor_scalar(
        acc[:], cf1[:, C_CWB : C_CWB + 33], gcol[:, 0:1], None, op0=OP.mult
    )
    term = sm.tile([128, 33], f32)
    for e in range(1, E):
        c0 = C_CWB + 33 * e
        nc.vector.tensor_scalar(
            term[:], cf1[:, c0 : c0 + 33], gcol[:, e : e + 1], None, op0=OP.mult
        )
        if e < E - 1:
            nc.vector.tensor_add(acc[:], acc[:], term[:])
        else:
            nc.vector.tensor_add(WB[:], acc[:], term[:])
    # block-diagonal bf16 lhsT via 4 partition-aligned copies (no bounce)
    nc.vector.memset(weTd[:].bitcast(f32), 0.0)
    for i in range(4):
        nc.vector.tensor_copy(
            weTd[32 * i : 32 * i + 32, 32 * i : 32 * i + 32],
            WB[32 * i : 32 * i + 32, 0:32],
        )

    # ---- main loop: per 512-position tile, conv both pairs into one PSUM
    # bank, one [128, n] tanh, 4 concurrent diag combine sub-matmuls.
    beff = WB[:, 32:33].bitcast(f32)
    for m in range(NT):
        c0 = m * TS
        n = min(TS, LP - c0)
        hp = psum_h.tile([128, TS], f32, tag="hp")
        for p in range(2):
            x0 = 2 * BW * m + BW * p
            for k in range(3):
                nc.tensor.matmul(
                    hp[64 * p : 64 * p + 64, 0:n],
                    cw1[:, 64 * k : 64 * k + 64],
                    xb[:, x0 + k : x0 + k + n],
                    start=(k == 0),
                    stop=(k == 2),
                )
        nc.scalar.activation(
            himg[:, c0 : c0 + n],
            hp[:, 0:n],
            AF.Tanh,
            bias=cf1[:, C_B1 : C_B1 + 1],
            scale=1.0,
        )
        yp = psum_o.tile([128, TS], f32, tag="op")
        nc.tensor.matmul(
            yp[:, 0:n], weTd[:], himg[:, c0 : c0 + n], start=True, stop=True
        )
        nc.vector.tensor_scalar(
            yimg[:, c0 : c0 + n], yp[:, 0:n], beff, None, op0=OP.add
        )
        # overlapped stores; yimg rows are (p, q, oc2), one DMA per pair
        # with a 3D DRAM AP (batch b = 2q+p).
        ydst = y_d.ap().rearrange("(q p) oc c -> p q oc c", q=2)
        STORES = {3: (0, 2048), 5: (2048, 3072), 6: (3072, 3584), 7: (3584, LP)}
        if m in STORES:
            a0, a1 = STORES[m]
            for p, eng in ((0, nc.scalar), (1, nc.sync)):
                eng.dma_start(
                    ydst[p : p + 1, :, :, a0:a1],
                    yimg[64 * p : 64 * p + 64, a0:a1],
                )


def _build():
    if "nc" in _CACHE:
        return _CACHE["nc"]
    nc = bacc.Bacc(
        "TRN2",
        target_bir_lowering=False,
        debug=False,
        num_devices=NCORES,
        detect_race_conditions=False,
    )
    f32 = mybir.dt.float32
    xb_d = nc.dram_tensor("xb", [128, XW], BF16, kind="ExternalInput")
    cf1_d = nc.dram_tensor("cf1", [128, NCF1], f32, kind="ExternalInput")
    cw1_d = nc.dram_tensor("cw1", [128, NW1], BF16, kind="ExternalInput")
    y_d = nc.dram_tensor("y", [NB, OC, LP], f32, kind="ExternalOutput")

    with tile.TileContext(nc) as tc:
        with ExitStack() as ctx:
            _emit(ctx, tc, nc, xb_d, cf1_d, cw1_d, y_d)
    nc.compile()
    _CACHE["nc"] = nc
    return nc


def _prep_shared(w_gate, conv1_w, conv1_b, conv2_w, conv2_b):
    import ml_dtypes

    bf = ml_dtypes.bfloat16
    w_gate = np.asarray(w_gate, np.float32)
    conv1_w = np.asarray(conv1_w, np.float32)
    conv1_b = np.asarray(conv1_b, np.float32)
    conv2_w = np.asarray(conv2_w, np.float32)
    conv2_b = np.asarray(conv2_b, np.float32)
    # cw1: block-diagonal conv1 lhsT, bf16
    cw1 = np.zeros((128, NW1), np.float32)
    wkt = conv1_w.transpose(1, 2, 0)  # [d, k, oc]
    for k in range(3):
        cw1[0:D, 64 * k : 64 * k + OC] = wkt[:, k, :]
        cw1[D : 2 * D, 64 * k + OC : 64 * k + 2 * OC] = wkt[:, k, :]
    cw1 = np.ascontiguousarray(cw1.astype(bf))
    # cf1 weight part (gate window filled per core)
    cf1w = np.zeros((128, NCF1), np.float32)
    wgr = w_gate.reshape(D, 5 * E)
    cf1w[0:D, C_WG : C_WG + 5 * E] = wgr
    cf1w[D : 2 * D, C_WG : C_WG + 5 * E] = wgr
    cf1w[:, C_B1] = np.tile(conv1_b, 4)
    # indicators (i-major): IND_q[p, 64p'+32q'+r] = d(p,p') d(q,q')
    for q in range(2):
        for p in range(2):
            r0 = 64 * p + 32 * q
            cf1w[p, C_IND + 128 * q + r0 : C_IND + 128 * q + r0 + 32] = 1.0
    # cwb block [32, 264]: cwb[ic, 33e+oc2] = conv2_w[oc2*E+e, ic, 0];
    #                      cwb[r, 33e+32]   = conv2_b[r*E+e]
    blk = np.zeros((32, 33 * E), np.float32)
    c2 = conv2_w[:, :, 0].reshape(OC, E, OC)  # [oc2, e, ic]
    c2b = conv2_b.reshape(OC, E)  # [oc2, e]
    for e in range(E):
        blk[:, 33 * e : 33 * e + 32] = c2[:, e, :].T  # [ic, oc2]
        blk[:, 33 * e + 32] = c2b[:, e]
    cf1w[:, C_CWB:] = np.tile(blk, (4, 1))
    return cw1, cf1w


def _prep_x(xc):
    """xc [4, D, L] f32 -> bf16 [128, XW] image.

    row 64q+d, col 2*BW*m + BW*p + j = x[2q+p, d, min(512m+j, L-1)].
    """
    import ml_dtypes

    bf = ml_dtypes.bfloat16
    xr = xc.reshape(2, 2, D, L).transpose(0, 2, 1, 3).reshape(128, 2, L)
    pos = np.minimum(512 * np.arange(NT)[:, None] + np.arange(BW)[None, :], L - 1)
    g = xr[:, :, pos]  # [128, p, m, j]
    return np.ascontiguousarray(
        g.transpose(0, 2, 1, 3).reshape(128, XW).astype(bf)
    )


def _run(x, w_gate, conv1_w, conv1_b, conv2_w, conv2_b, **spmd_kwargs):
    x = np.asarray(x, np.float32)
    assert x.shape == (B, D, L), x.shape
    cw1, cf1w = _prep_shared(w_gate, conv1_w, conv1_b, conv2_w, conv2_b)
    nc = _build()
    in_maps = []
    for i in range(NCORES):
        xc = x[NB * i : NB * (i + 1)]  # [4, D, L]
        cf1 = cf1w.copy()
        g4 = xc[:, :, L - 6 : L - 1].reshape(2, 2, D, 5)  # [q, p, d, t]
        cf1[:, C_GW : C_GW + 10] = g4.transpose(0, 2, 3, 1).reshape(128, 10)
        in_maps.append({"xb": _prep_x(xc), "cf1": cf1, "cw1": cw1})
    res = bass_utils.run_bass_kernel_spmd(
        nc, in_maps, core_ids=list(range(NCORES)), **spmd_kwargs
    )
    y = np.concatenate([r["y"] for r in res.results], axis=0)
    return np.ascontiguousarray(y.astype(np.float32)), res


def kernel(x, w_gate, conv1_w, conv1_b, conv2_w, conv2_b):
    y, _ = _run(x, w_gate, conv1_w, conv1_b, conv2_w, conv2_b)
    return y


# revision 19
# speedup vs baseline: 1.9481x; 1.9481x over previous
"""MoE-routed conv kernel (Channel_Embedding ablation) for 8 trn2 NeuronCores.

Math (see reference):
  gates  = top2-renormalized softmax( x[:, :, -6:-1].reshape(B, D*5) @ w_gate )
  h      = tanh(conv1d(x, conv1_w, VALID) + conv1_b)            # [B, OC, L-2]
  out    = conv1d(h, conv2_w, 1x1) + conv2_b                    # [B, OC*E, L-2]
  y[b,oc,t] = sum_e gates[b,e] * out[b, oc*E+e, t]

Key algebraic fold: the expert combine commutes with the 1x1 conv, so per
batch element y[b] = W_eff[b] @ h[b] + b_eff[b] with
  W_eff[b] = sum_e gates[b,e] * conv2_w[:,:,0][oc*E+e, :],
  b_eff[b] = sum_e gates[b,e] * conv2_b[oc*E+e].

Sharding: data-parallel over batch B=32 across 8 cores (4 each, b=2q+p).

v3 design (trace-driven; see v1/v2 history in git-less comments):
  * Host pre-packs x into a bf16 [128, 8*1028] image whose column blocks
    interleave the two batch pairs per 512-position tile, so each tile's
    conv runs p0 into PSUM rows 0:64 and p1 into rows 64:128 of ONE bank
    => ONE [128, n] tanh per tile (ACT cost is per-column, so pairing
    halves activation work vs per-pair tanh).
  * The gating window is duplicated into the f32 const image: gating
    runs in exact f32 and never touches the bf16 x image.
  * NO W_eff DRAM bounce (v2's combine stalled ~12us on that DMA chain).
    Instead: gates are block-broadcast to a [128, 8] column via two tiny
    indicator matmuls, and the combine lhsT+bias image WB[128, 33] is
    built on the DVE as sum_e gcol[:, e] * cwb[:, 33e:33e+33] from a
    host-packed per-expert image. WB rows (p, q, *) hold W_eff[2q+p].T
    diag-blocks (cols 0:32) and b_eff (col 32).
  * Combine per tile = 4 CONCURRENT 32x32 sub-matmuls via tile_position
    (32i, 32i) -- span of one matmul for all four batches.
  * y accumulates in a [128=(p,q,oc2), LP] f32 image; stores are 4 plain
    DMAs with a 4D DRAM-side AP, overlapped with the loop.
  * DMA rings: x chunks on sync, consts + stores on scalar; only 11
    dma_starts total (v1 had 40, all on sync, which serialized issue).
"""

from contextlib import ExitStack

import numpy as np

import concourse.bacc as bacc
import concourse.mybir as mybir
import concourse.tile as tile
from concourse import bass_utils

B, D, L = 32, 64, 4096
E, TOPK, OC = 8, 2, 32
LP = L - 2  # 4094 valid conv outputs
NCORES = 8
NB = B // NCORES  # batch elements per core
TS = 512  # position tile (one PSUM bank of fp32)
NT = (LP + TS - 1) // TS
BW = 2 + TS  # x image block half-width (conv needs +2 cols)
XW = NT * 2 * BW  # x image width: per tile, p0 block then p1 block

BF16 = mybir.dt.bfloat16
FAST_DT = mybir.dt.float32r

# cf1 [128, NCF1] f32 column map
C_GW = 0  # [*, 10] gate window, col = 2t+p, row = 64q+d
C_WG = 10  # [*, 40] w_gate, col = 8t+e (dup in both q halves)
C_B1 = 50  # [*, 1] conv1 bias tiled 4x, row = 32j+oc
C_IND = 51  # [0:2, 256] block-broadcast indicators (i-major), 128 per q
C_INDB = C_IND + 256  # [0:2, 256] same, b-major rows (for the bias build)
C_CWB = C_INDB + 256  # [*, 264] cwb[32j+r, 33e+*]: conv2 expert image
NCF1 = C_CWB + 33 * E
NW1 = 3 * 2 * OC  # cw1 [128, 192] bf16 block-diag conv1 lhsT

_CACHE: dict = {}


def _softmax_top2(nc, sm, lg, f32, AX, OP, AF, q):
    """Per-half gating: lg [2, E] logits (PSUM) -> gates [2, E] in SBUF.

    gates = (e >= m2) * e / (m1 + m2 + 1e-6 * sum(e)), e = exp(logits) --
    identical to softmax -> top2 -> vk/(sum vk + 1e-6) in exact arithmetic.
    Returns gpad [32, 32] with gates for batches {2q, 2q+1} at [0:2, 0:E].
    """
    e_sb = sm.tile([2, E], f32, name=f"e_sb{q}")
    nc.scalar.activation(e_sb[:], lg[:], AF.Exp)
    m1 = sm.tile([2, 1], f32, name=f"m1_{q}")
    nc.vector.reduce_max(m1[:], e_sb[:], axis=AX.X)
    lt = sm.tile([2, E], f32, name=f"lt{q}")
    nc.vector.tensor_scalar(lt[:], e_sb[:], m1[:], None, op0=OP.is_lt)
    emsk = sm.tile([2, E], f32, name=f"emsk{q}")
    nc.vector.tensor_mul(emsk[:], lt[:], e_sb[:])  # e with the max zeroed
    m2 = sm.tile([2, 1], f32, name=f"m2_{q}")
    nc.vector.reduce_max(m2[:], emsk[:], axis=AX.X)
    den3 = sm.tile([2, 1], f32, name=f"den3{q}")
    nc.vector.tensor_add(den3[:], m1[:], m2[:])
    rcp = sm.tile([2, 1], f32, name=f"rcp{q}")
    nc.vector.reciprocal(rcp[:], den3[:])
    ge = sm.tile([2, E], f32, name=f"ge{q}")
    nc.vector.tensor_scalar(ge[:], e_sb[:], m2[:], None, op0=OP.is_ge)
    gnum = sm.tile([2, E], f32, name=f"gnum{q}")
    nc.vector.tensor_mul(gnum[:], ge[:], e_sb[:])
    gpad = sm.tile([32, 32], f32, name=f"gpad{q}")
    nc.vector.memset(gpad[:], 0.0)
    nc.vector.tensor_scalar(gpad[0:2, 0:E], gnum[:], rcp[:], None, op0=OP.mult)
    return gpad  # gpad[p, e] = gates[2q+p, e]


def _emit(ctx, tc, nc, xb_d, cf1_d, cw1_d, y_d):
    f32 = mybir.dt.float32
    AF = mybir.ActivationFunctionType
    AX = mybir.AxisListType
    OP = mybir.AluOpType

    const = ctx.enter_context(tc.tile_pool(name="const", bufs=1))
    sm = ctx.enter_context(tc.tile_pool(name="sm", bufs=1))
    psum_h = ctx.enter_context(tc.tile_pool(name="ph", bufs=3, space="PSUM"))
    psum_o = ctx.enter_context(tc.tile_pool(name="po", bufs=2, space="PSUM"))
    psum_s = ctx.enter_context(tc.tile_pool(name="ps", bufs=2, space="PSUM"))

    # ---- persistent images
    xb = const.tile([128, XW], BF16)
    cf1 = const.tile([128, NCF1], f32)
    cw1 = const.tile([128, NW1], BF16)
    WB = const.tile([128, 33], FAST_DT)  # W_eff.T diag blocks + b_eff col
    weTd = const.tile([128, 128], BF16)  # block-diag combine lhsT
    gcol = const.tile([128, E], f32)
    himg = const.tile([128, LP], BF16)  # h, row = 64p+32q+oc1
    yimg = const.tile([128, LP], f32)  # y, row = 64p+32q+oc2

    # ---- DMA issue: consts on the scalar ring, x chunks on the sync ring
    nc.scalar.dma_start(cf1[:], cf1_d.ap())
    nc.scalar.dma_start(cw1[:], cw1_d.ap())
    XCH = [0, BW, 2 * BW, 4 * BW, 6 * BW, 8 * BW, 10 * BW, 12 * BW, 16 * BW]
    for a0, a1 in zip(XCH[:-1], XCH[1:]):
        nc.sync.dma_start(xb[:, a0:a1], xb_d.ap()[:, a0:a1])

    # ---- ACT table warmup (exp/tanh share one table set; load it early)
    warm = sm.tile([1, 8], f32)
    nc.vector.memset(warm[:], 0.0)
    warm2 = sm.tile([1, 8], f32)
    nc.scalar.activation(warm2[:], warm[:], AF.Exp)

    # ---- PE warmup: bf16 dummy matmuls bridge PE activity from t=0 into
    # the first real matmuls so the power state ramps during the load.
    wsrc = sm.tile([128, 256], f32)
    nc.vector.memset(wsrc[:], 0.0)
    wsb = wsrc[:].bitcast(BF16)  # [128, 512] of zeros
    for _ in range(4):
        wup = psum_h.tile([128, TS], f32, tag="hp")
        nc.tensor.matmul(wup[:], wsb[:, 0:128], wsb[:], start=True, stop=True)

    # ---- gating from the f32 const image (exact f32, batches b=2q+p)
    gpads = []
    for q in range(2):
        lg = psum_s.tile([2, E], f32, tag="s", name=f"lg{q}")
        for t in range(5):
            nc.tensor.matmul(
                lg[:],
                cf1[D * q : D * q + D, C_GW + 2 * t : C_GW + 2 * t + 2],
                cf1[D * q : D * q + D, C_WG + E * t : C_WG + E * t + E],
                start=(t == 0),
                stop=(t == 4),
            )
        gpads.append(_softmax_top2(nc, sm, lg, f32, AX, OP, AF, q))

    # ---- block-broadcast gates: gcol[64p+32q+r, e] = gates[2q+p, e]
    gcp = psum_s.tile([128, E], f32, tag="s")
    for q in range(2):
        nc.tensor.matmul(
            gcp[:],
            cf1[0:2, C_IND + 128 * q : C_IND + 128 * q + 128],
            gpads[q][0:2, 0:E],
            start=(q == 0),
            stop=(q == 1),
        )
    nc.vector.tensor_copy(gcol[:], gcp[:])

    # ---- WB = sum_e gcol[:, e] * cwb_e  (DVE build; no DRAM bounce)
    acc = sm.tile([128, 33], f32)
    nc.vector.tensor_scalar(
        acc[:], cf1[:, C_CWB : C_CWB + 33], gcol[:, 0:1], None, op0=OP.mult
    )
    term = sm.tile([128, 33], f32)
    for e in range(1, E):
        c0 = C_CWB + 33 * e
        nc.vector.tensor_scalar(
            term[:], cf1[:, c0 : c0 + 33], gcol[:, e : e + 1], None, op0=OP.mult
        )
        if e < E - 1:
            nc.vector.tensor_add(acc[:], acc[:], term[:])
        else:
            nc.vector.tensor_add(WB[:], acc[:], term[:])
    # block bf16 lhsT via 4 partition-aligned copies (no DRAM bounce);
    # column offset 32b makes the combine OUTPUT b-major (himg block
    # i = 2p+q holds batch b = 2q+p), so y stores are a plain 2D merge.
    nc.vector.memset(weTd[:].bitcast(f32), 0.0)
    for i in range(4):
        b = 2 * (i % 2) + i // 2
        nc.vector.tensor_copy(
            weTd[32 * i : 32 * i + 32, 32 * b : 32 * b + 32],
            WB[32 * i : 32 * i + 32, 0:32],
        )
    # b-ordered bias column for the drain
    gcpb = psum_s.tile([128, E], f32, tag="s", name="gcpb")
    for q in range(2):
        nc.tensor.matmul(
            gcpb[:],
            cf1[0:2, C_INDB + 128 * q : C_INDB + 128 * q + 128],
            gpads[q][0:2, 0:E],
            start=(q == 0),
            stop=(q == 1),
        )
    gcolb = sm.tile([128, E], f32, name="gcolb")
    nc.vector.tensor_copy(gcolb[:], gcpb[:])
    biasb = const.tile([128, 1], f32)
    accb = sm.tile([128, 1], f32)
    nc.vector.tensor_scalar(
        accb[:], cf1[:, C_CWB + 32 : C_CWB + 33], gcolb[:, 0:1], None, op0=OP.mult
    )
    termb = sm.tile([128, 1], f32)
    for e in range(1, E):
        cb = C_CWB + 33 * e + 32
        nc.vector.tensor_scalar(
            termb[:], cf1[:, cb : cb + 1], gcolb[:, e : e + 1], None, op0=OP.mult
        )
        nc.vector.tensor_add(biasb[:] if e == E - 1 else accb[:], accb[:], termb[:])

    # ---- main loop: per 512-position tile, conv both pairs into one PSUM
    # bank, one [128, n] tanh, 4 concurrent diag combine sub-matmuls.
    beff = WB[:, 32:33].bitcast(f32)
    for m in range(NT):
        c0 = m * TS
        n = min(TS, LP - c0)
        hp = psum_h.tile([128, TS], f32, tag="hp")
        for p in range(2):
            x0 = 2 * BW * m + BW * p
            for k in range(3):
                nc.tensor.matmul(
                    hp[64 * p : 64 * p + 64, 0:n],
                    cw1[:, 64 * k : 64 * k + 64],
                    xb[:, x0 + k : x0 + k + n],
                    start=(k == 0),
                    stop=(k == 2),
                )
        nc.scalar.activation(
            himg[:, c0 : c0 + n],
            hp[:, 0:n],
            AF.Tanh,
            bias=cf1[:, C_B1 : C_B1 + 1],
            scale=1.0,
        )
        yp = psum_o.tile([128, TS], f32, tag="op")
        nc.tensor.matmul(
            yp[:, 0:n], weTd[:], himg[:, c0 : c0 + n], start=True, stop=True
        )
        nc.vector.tensor_scalar(
            yimg[:, c0 : c0 + n], yp[:, 0:n], biasb[:, 0:1], None, op0=OP.add
        )
        # overlapped stores; DRAM side merges (b oc) into the partition dim
        ydst = y_d.ap().rearrange("b oc c -> (b oc) c")
        if m == 3:
            nc.scalar.dma_start(ydst[:, 0:2048], yimg[:, 0:2048])
        elif m == 5:
            nc.sync.dma_start(ydst[:, 2048:3072], yimg[:, 2048:3072])
        elif m == 6:
            nc.scalar.dma_start(ydst[:, 3072:3584], yimg[:, 3072:3584])
        elif m == 7:
            nc.sync.dma_start(ydst[:, 3584:LP], yimg[:, 3584:LP])


def _build():
    if "nc" in _CACHE:
        return _CACHE["nc"]
    nc = bacc.Bacc(
        "TRN2",
        target_bir_lowering=False,
        debug=False,
        num_devices=NCORES,
        detect_race_conditions=False,
    )
    f32 = mybir.dt.float32
    xb_d = nc.dram_tensor("xb", [128, XW], BF16, kind="ExternalInput")
    cf1_d = nc.dram_tensor("cf1", [128, NCF1], f32, kind="ExternalInput")
    cw1_d = nc.dram_tensor("cw1", [128, NW1], BF16, kind="ExternalInput")
    y_d = nc.dram_tensor("y", [NB, OC, LP], f32, kind="ExternalOutput")

    with tile.TileContext(nc) as tc:
        with ExitStack() as ctx:
            _emit(ctx, tc, nc, xb_d, cf1_d, cw1_d, y_d)
    nc.compile()
    _CACHE["nc"] = nc
    return nc


def _prep_shared(w_gate, conv1_w, conv1_b, conv2_w, conv2_b):
    import ml_dtypes

    bf = ml_dtypes.bfloat16
    w_gate = np.asarray(w_gate, np.float32)
    conv1_w = np.asarray(conv1_w, np.float32)
    conv1_b = np.asarray(conv1_b, np.float32)
    conv2_w = np.asarray(conv2_w, np.float32)
    conv2_b = np.asarray(conv2_b, np.float32)
    # cw1: block-diagonal conv1 lhsT, bf16
    cw1 = np.zeros((128, NW1), np.float32)
    wkt = conv1_w.transpose(1, 2, 0)  # [d, k, oc]
    for k in range(3):
        cw1[0:D, 64 * k : 64 * k + OC] = wkt[:, k, :]
        cw1[D : 2 * D, 64 * k + OC : 64 * k + 2 * OC] = wkt[:, k, :]
    cw1 = np.ascontiguousarray(cw1.astype(bf))
    # cf1 weight part (gate window filled per core)
    cf1w = np.zeros((128, NCF1), np.float32)
    wgr = w_gate.reshape(D, 5 * E)
    cf1w[0:D, C_WG : C_WG + 5 * E] = wgr
    cf1w[D : 2 * D, C_WG : C_WG + 5 * E] = wgr
    cf1w[:, C_B1] = np.tile(conv1_b, 4)
    # indicators: i-major (WB/weTd rows) and b-major (bias rows)
    for q in range(2):
        for p in range(2):
            r0 = 64 * p + 32 * q
            cf1w[p, C_IND + 128 * q + r0 : C_IND + 128 * q + r0 + 32] = 1.0
            rb = 32 * (2 * q + p)
            cf1w[p, C_INDB + 128 * q + rb : C_INDB + 128 * q + rb + 32] = 1.0
    # cwb block [32, 264]: cwb[ic, 33e+oc2] = conv2_w[oc2*E+e, ic, 0];
    #                      cwb[r, 33e+32]   = conv2_b[r*E+e]
    blk = np.zeros((32, 33 * E), np.float32)
    c2 = conv2_w[:, :, 0].reshape(OC, E, OC)  # [oc2, e, ic]
    c2b = conv2_b.reshape(OC, E)  # [oc2, e]
    for e in range(E):
        blk[:, 33 * e : 33 * e + 32] = c2[:, e, :].T  # [ic, oc2]
        blk[:, 33 * e + 32] = c2b[:, e]
    cf1w[:, C_CWB:] = np.tile(blk, (4, 1))
    return cw1, cf1w


def _prep_x(xc):
    """xc [4, D, L] f32 -> bf16 [128, XW] image.

    row 64q+d, col 2*BW*m + BW*p + j = x[2q+p, d, min(512m+j, L-1)].
    """
    import ml_dtypes

    bf = ml_dtypes.bfloat16
    xr = xc.reshape(2, 2, D, L).transpose(0, 2, 1, 3).reshape(128, 2, L)
    pos = np.minimum(512 * np.arange(NT)[:, None] + np.arange(BW)[None, :], L - 1)
    g = xr[:, :, pos]  # [128, p, m, j]
    return np.ascontiguousarray(
        g.transpose(0, 2, 1, 3).reshape(128, XW).astype(bf)
    )


def _run(x, w_gate, conv1_w, conv1_b, conv2_w, conv2_b, **spmd_kwargs):
    x = np.asarray(x, np.float32)
    assert x.shape == (B, D, L), x.shape
    cw1, cf1w = _prep_shared(w_gate, conv1_w, conv1_b, conv2_w, conv2_b)
    nc = _build()
    in_maps = []
    for i in range(NCORES):
        xc = x[NB * i : NB * (i + 1)]  # [4, D, L]
        cf1 = cf1w.copy()
        g4 = xc[:, :, L - 6 : L - 1].reshape(2, 2, D, 5)  # [q, p, d, t]
        cf1[:, C_GW : C_GW + 10] = g4.transpose(0, 2, 3, 1).reshape(128, 10)
        in_maps.append({"xb": _prep_x(xc), "cf1": cf1, "cw1": cw1})
    res = bass_utils.run_bass_kernel_spmd(
        nc, in_maps, core_ids=list(range(NCORES)), **spmd_kwargs
    )
    y = np.concatenate([r["y"] for r in res.results], axis=0)
    return np.ascontiguousarray(y.astype(np.float32)), res


def kernel(x, w_gate, conv1_w, conv1_b, conv2_w, conv2_b):
    y, _ = _run(x, w_gate, conv1_w, conv1_b, conv2_w, conv2_b)
    return y
